# revision 30
# baseline (speedup 1.0000x reference)
"""Trainium2 Bass kernel for nn_FactorCovModel.

Model: 2-layer LSTM (H=512) over [B=256, T=64, D=500], last hidden ->
FC [512 -> 16532] -> Sigma = Lambda diag(exp(fv)) Lambda^T + diag(exp(idio)),
output [256, 500, 500].

Sharding: pure data parallel over batch, 32 samples/core on 8 cores.

Per-core design (matmul operands bf16, fp32 PSUM accumulation):
  - Gate axis host-permuted to [i, f, o, g] x hidden-group so PSUM col
    group hg holds hidden slice hg of all four gates; sigmoid covers one
    contiguous [128, 384] op, tanh one [128, 128] op.
  - LSTM gates col-tiled: stationary = x/hT chunk [128, 32] at positions
    (0, 32j); 4 hidden-group strips run concurrently, one PSUM bank each.
  - l1 bias injected via a K=1 ones-row matmul (start of the accum group)
    instead of DVE adds during evac.
  - Emission is software-pipelined: per t we emit l1-MMs(t-1), l0-MMs(t),
    l1-elementwise(t-1), l0-elementwise(t) so the PE never starves while
    ACT/DVE run the nonlinearity of the other layer.
  - Evacs split 2-on-ACT / 2-on-DVE; elementwise kept bf16 where the c
    accumulator doesn't need fp32.
  - FC bias also injected via K=1 matmul; Lambda blocks PE-transposed per
    n-tile into one PSUM bank -> one [32, 512] cast into LT; LT then
    DMA-replicated to partition offsets 32/64/96 so per-sample Sigma runs
    4 concurrent row-tiled matmuls (one per 128-row m-tile).
  - Sigma written to DRAM as bf16 (halves the 32 MB/core output traffic);
    host converts to fp32, mirrors nothing (full square written), applies
    exp to idio raw rows and adds the diagonal.
"""

import os
import sys

sys.path.insert(0, "/opt/trn_rl_repo")

import numpy as np

import concourse.bass as bass
import concourse.mybir as mybir
from concourse import bacc
from concourse.tile import TileContext

FP = mybir.dt.float32
BF = mybir.dt.bfloat16
AF = mybir.ActivationFunctionType

B_FULL, T_FULL, D_IN, H = 256, 64, 500, 512
NCORES = 8
BL = B_FULL // NCORES            # 32 samples per core
NA, NF = 500, 32                 # assets, factors
OUT_DIM = NA * NF + NF + NA      # 16532
NTILE = 512                      # FC feature tile
N_FTILES = 33                    # ceil(16532/512) -> features padded to 16896
FH = N_FTILES * NTILE            # 16896
XCHUNK = 16                      # time steps per streamed xT chunk
NPREF = 24                       # fcw tiles prefetched during the LSTM

# gate-axis permutation: new col (hg, gate', hl) = 512*hg + 128*gate' + hl maps
# to old row og*512 + 128*hg + hl with og = [i,f,o,g] -> torch [i,f,g,o] index.
# With this layout PSUM col group hg holds [i|f|o|g] x 128 lanes of hidden
# slice hg, so sigmoid is one [*, 0:384] op and tanh one [*, 384:512] op.
_OG = [0, 1, 3, 2]
PERM = np.array([_OG[g] * 512 + 128 * hg + hl
                 for hg in range(4) for g in range(4) for hl in range(128)])


# ---------------------------------------------------------------- host prep

def host_prep_shared(inputs):
    import ml_dtypes
    tobf = lambda a: np.ascontiguousarray(a, dtype=ml_dtypes.bfloat16)

    w_ih0 = np.asarray(inputs["w_ih0"])[PERM]
    w_hh0 = np.asarray(inputs["w_hh0"])[PERM]
    b0 = (np.asarray(inputs["b_ih0"]) + np.asarray(inputs["b_hh0"]))[PERM]
    w_ih1 = np.asarray(inputs["w_ih1"])[PERM]
    w_hh1 = np.asarray(inputs["w_hh1"])[PERM]
    b1 = (np.asarray(inputs["b_ih1"]) + np.asarray(inputs["b_hh1"]))[PERM]
    fc_w = np.asarray(inputs["fc_w"])
    fc_b = np.asarray(inputs["fc_b"])

    w0T = np.zeros((512, 2048), np.float32)
    w0T[:500] = w_ih0.T
    w0T[500] = b0
    wh0T = np.ascontiguousarray(w_hh0.T, dtype=np.float32)
    w1T = np.ascontiguousarray(np.concatenate([w_ih1.T, w_hh1.T]), dtype=np.float32)
    b1row = b1.reshape(1, 2048)
    fcwT = np.zeros((512, FH), np.float32)
    fcwT[:, :OUT_DIM] = fc_w.T
    fcbRow = np.zeros((1, FH), np.float32)
    fcbRow[0, :OUT_DIM] = fc_b
    onesT = np.ones((1, 32), np.float32)
    ident = np.ascontiguousarray(np.tile(np.eye(32, dtype=np.float32), (4, 1)))
    identb = ident.copy()
    return dict(w0T=tobf(w0T), wh0T=tobf(wh0T), w1T=tobf(w1T),
                b1row=tobf(b1row), fcwT=tobf(fcwT), fcbRow=tobf(fcbRow),
                onesT=tobf(onesT), identt=np.ascontiguousarray(ident),
                identb=tobf(identb))


def host_prep_x(x_core):
    """x_core [BL, T, 500] -> xT [512, T*BL], (t, b) free order, ones bias row."""
    T = x_core.shape[1]
    import ml_dtypes
    xT = np.zeros((512, T * BL), np.float32)
    xT[:500] = np.asarray(x_core, np.float32).transpose(2, 1, 0).reshape(500, T * BL)
    xT[500] = 1.0
    return np.ascontiguousarray(xT, dtype=ml_dtypes.bfloat16)


# ---------------------------------------------------------------- bass build

def build_nc(T=T_FULL):
    nc = bacc.Bacc("TRN2")

    xT_d = nc.dram_tensor("xT", [512, T * BL], BF, kind="ExternalInput")
    w0T_d = nc.dram_tensor("w0T", [512, 2048], BF, kind="ExternalInput")
    wh0T_d = nc.dram_tensor("wh0T", [512, 2048], BF, kind="ExternalInput")
    w1T_d = nc.dram_tensor("w1T", [1024, 2048], BF, kind="ExternalInput")
    b1row_d = nc.dram_tensor("b1row", [1, 2048], BF, kind="ExternalInput")
    fcwT_d = nc.dram_tensor("fcwT", [512, FH], BF, kind="ExternalInput")
    fcbRow_d = nc.dram_tensor("fcbRow", [1, FH], BF, kind="ExternalInput")
    onesT_d = nc.dram_tensor("onesT", [1, 32], BF, kind="ExternalInput")
    identt_d = nc.dram_tensor("identt", [128, 32], FP, kind="ExternalInput")
    identb_d = nc.dram_tensor("identb", [128, 32], BF, kind="ExternalInput")

    sigma_d = nc.dram_tensor("sigma", [BL, NA, NA], BF, kind="ExternalOutput")
    idio_d = nc.dram_tensor("idio_raw", [BL, NA], FP, kind="ExternalOutput")

    def mm(out, lhsT, rhs, tp, **kw):
        nc.tensor.matmul(out, lhsT, rhs,
                         tile_position=tp, skip_group_check=True, **kw)

    def tr(out, in_, identity, tp):
        nc.tensor.matmul(out, in_, identity, is_transpose=True,
                         tile_position=tp, skip_group_check=True)

    with TileContext(nc) as tc:
        with tc.tile_pool(name="persist", bufs=1) as persist:
            b1row_sb = persist.tile([1, 2048], BF)
            nc.sync.dma_start(b1row_sb, b1row_d[:, :])
            ones_sb = persist.tile([1, 32], BF)
            nc.sync.dma_start(ones_sb, onesT_d[:, :])
            identt_sb = persist.tile([128, 32], FP)
            nc.sync.dma_start(identt_sb, identt_d[:, :])
            identb_sb = persist.tile([128, 32], BF)
            nc.sync.dma_start(identb_sb, identb_d[:, :])
            hlast = persist.tile([128, 128], BF)  # final h1T, chunk-major cols
            # fcw prefetch tile: loaded AFTER the LSTM weights are issued so
            # the 12 MB transfer doesn't head-of-line block the first step
            fcw_pref = persist.tile([128, 4, NPREF * 512], BF)

            # ---------------- phase 1: LSTM ----------------
            with (
                tc.tile_pool(name="wconst", bufs=1) as wconst,
                tc.tile_pool(name="xring", bufs=2) as xring,
                tc.tile_pool(name="state", bufs=2) as state,
                tc.tile_pool(name="work", bufs=2) as work,
                tc.tile_pool(name="p0", bufs=4, space="PSUM") as p0,
                tc.tile_pool(name="p1", bufs=4, space="PSUM") as p1,
            ):
                w0T_sb = wconst.tile([128, 4, 2048], BF)
                wh0T_sb = wconst.tile([128, 4, 2048], BF)
                w1T_sb = wconst.tile([128, 8, 2048], BF)

                xch = min(XCHUNK, T)
                n_xchunks = (T + xch - 1) // xch
                x_tiles = {}

                def load_xchunk(ci):
                    if ci >= n_xchunks or ci in x_tiles:
                        return
                    xt = xring.tile([128, 4, xch * BL], BF, tag="xchunk")
                    nc.sync.dma_start(
                        xt,
                        xT_d[:, ci * xch * BL:(ci + 1) * xch * BL]
                        .rearrange("(ko p) tb -> p ko tb", p=128),
                    )
                    x_tiles[ci] = xt

                # x chunk 0 first (smallest critical load), then weights in
                # consumption order, then the bulky fcw prefetch
                load_xchunk(0)
                nc.sync.dma_start(w0T_sb, w0T_d.rearrange("(ko p) g -> p ko g", p=128))
                nc.sync.dma_start(wh0T_sb, wh0T_d.rearrange("(ko p) g -> p ko g", p=128))
                nc.sync.dma_start(w1T_sb, w1T_d.rearrange("(ko p) g -> p ko g", p=128))
                load_xchunk(1)
                nc.sync.dma_start(
                    fcw_pref,
                    fcwT_d[:, 0:NPREF * 512]
                    .rearrange("(ko p) n -> p ko n", p=128),
                )

                def mm_xproj(t):
                    """x-projection of step t into fresh l0 gate banks."""
                    ci, tl = t // xch, t % xch
                    if tl == 0:
                        load_xchunk(ci + 1)
                    xt = x_tiles[ci]
                    pgs = [p0.tile([128, 512], FP, tag="g", name=f"g0_{t}_{j}")
                           for j in range(4)]
                    for k in range(4):
                        lhsT = xt[:, k, tl * BL:(tl + 1) * BL]
                        for j in range(4):
                            mm(pgs[j][32 * j:32 * (j + 1), :], lhsT,
                               w0T_sb[:, k, 512 * j:512 * (j + 1)],
                               tp=(0, 32 * j),
                               start=(k == 0), stop=(t == 0 and k == 3))
                    return pgs

                def mm_rec(t, pgs):
                    """recurrent accumulation of layer 0, step t."""
                    h0T = st["h0T"]
                    for k in range(4):
                        lhsT = h0T[:, 32 * k:32 * (k + 1)]
                        for j in range(4):
                            mm(pgs[j][32 * j:32 * (j + 1), :], lhsT,
                               wh0T_sb[:, k, 512 * j:512 * (j + 1)],
                               tp=(0, 32 * j),
                               start=False, stop=(k == 3))

                def mm_l1a(t):
                    """bias + h0T(t) half of layer 1's accumulation."""
                    pgs = [p1.tile([128, 512], FP, tag="g", name=f"g1_{t}_{j}")
                           for j in range(4)]
                    for j in range(4):
                        mm(pgs[j][32 * j:32 * (j + 1), :], ones_sb[0:1, :],
                           b1row_sb[0:1, 512 * j:512 * (j + 1)],
                           tp=(0, 32 * j), start=True, stop=False)
                    h0T = st["h0T"]
                    for k in range(4):
                        lhsT = h0T[:, 32 * k:32 * (k + 1)]
                        for j in range(4):
                            mm(pgs[j][32 * j:32 * (j + 1), :], lhsT,
                               w1T_sb[:, k, 512 * j:512 * (j + 1)],
                               tp=(0, 32 * j),
                               start=False, stop=(t == 0 and k == 3))
                    return pgs

                def mm_l1b(t, pgs):
                    """h1T(t-1) half of layer 1's accumulation (t >= 1)."""
                    h1T = st["h1T"]
                    for k in range(4):
                        lhsT = h1T[:, 32 * k:32 * (k + 1)]
                        for j in range(4):
                            mm(pgs[j][32 * j:32 * (j + 1), :], lhsT,
                               w1T_sb[:, 4 + k, 512 * j:512 * (j + 1)],
                               tp=(0, 32 * j),
                               start=False, stop=(k == 3))

                st = {"h0T": None, "h1T": None, "c0": None, "c1": None}

                def elem_nl(t, layer, pgs):
                    """evac + nonlinearity + DMA-xbar transpose -> hT."""
                    tag = f"l{layer}"
                    a = work.tile([128, 512], FP, tag=f"a_{tag}")
                    for j in range(4):
                        s = slice(32 * j, 32 * (j + 1))
                        if j % 2 == 0:
                            nc.scalar.copy(a[s, :], pgs[j][s, :])
                        else:
                            nc.vector.tensor_copy(a[s, :], pgs[j][s, :])
                    act = work.tile([128, 512], FP, tag=f"act_{tag}")
                    nc.scalar.activation(act[:, 0:384], a[:, 0:384], AF.Sigmoid)
                    nc.scalar.activation(act[:, 384:512], a[:, 384:512], AF.Tanh)
                    t1 = work.tile([128, 128], FP, tag=f"t1_{tag}")
                    nc.vector.tensor_mul(t1, act[:, 0:128], act[:, 384:512])
                    cprev = st[f"c{layer}"]
                    if cprev is None:
                        cn = t1
                    else:
                        t2 = work.tile([128, 128], FP, tag=f"t2_{tag}")
                        nc.vector.tensor_mul(t2, act[:, 128:256], cprev)
                        cn = work.tile([128, 128], FP, tag=f"c_{tag}")
                        nc.vector.tensor_add(cn, t1, t2)
                    st[f"c{layer}"] = cn
                    tcn = work.tile([128, 128], FP, tag=f"tc_{tag}")
                    nc.scalar.activation(tcn, cn, AF.Tanh)
                    hh = work.tile([128, 128], BF, tag=f"h_{tag}")
                    nc.vector.tensor_mul(hh, act[:, 256:384], tcn)
                    ht = state.tile([128, 128], BF, tag=f"ht_{tag}")
                    nc.sync.dma_start(ht, hh, transpose=True)
                    st[f"h{layer}T"] = ht

                def warmers(n):
                    # standalone LDWEIGHTS keep the PE's HAM activity monitor
                    # busy through the nonlinearity window (no PSUM side
                    # effects; every matmul reloads its own stationary)
                    for _ in range(n):
                        nc.tensor.ldweights(w0T_sb[:, 0, 0:128])

                # Software-pipelined emission; PE FIFO per step:
                #   rec(t) | l1-bias+h0T(t-1) | xproj(t+1) | l1-h1T(t-1) |
                #   warmers  -- while ACT/DVE run l0(t) then l1(t-1)
                # elementwise and the h transposes go through the DMA xbar.
                pgs0 = mm_xproj(0)
                pgs1 = None
                for t in range(T):
                    if t > 0:
                        mm_rec(t, pgs0)
                        pgs1 = mm_l1a(t - 1)
                    h0args = (t, 0, pgs0)
                    elem_nl(*h0args)
                    if t + 1 < T:
                        pgs0 = mm_xproj(t + 1)
                    if t > 1:
                        mm_l1b(t - 1, pgs1)
                    warmers(12)
                    if t > 0:
                        elem_nl(t - 1, 1, pgs1)
                pgs1 = mm_l1a(T - 1)
                mm_l1b(T - 1, pgs1)
                elem_nl(T - 1, 1, pgs1)
                nc.vector.tensor_copy(hlast, st["h1T"])

            # ---------------- phase 2: FC + Lambda layout + Sigma ----------------
            with (
                tc.tile_pool(name="fcw", bufs=3) as fcwp,
                tc.tile_pool(name="rawp", bufs=3) as rawp,
                tc.tile_pool(name="lt", bufs=1) as ltp,
                tc.tile_pool(name="gt", bufs=3) as gtp,
                tc.tile_pool(name="sigw", bufs=6) as sigw,
                tc.tile_pool(name="pfc", bufs=4, space="PSUM") as pfcp,
                tc.tile_pool(name="pp", bufs=4, space="PSUM") as ppp,
            ):
                # LT4: [128 = (rep q, factor f), 32 b, 500 assets] bf16
                LT4 = ltp.tile([128, 32, 512], BF)
                F4 = ltp.tile([128, 32], FP)   # exp(fvar raw + bias), 4 reps

                n_quads = (N_FTILES + 3) // 4          # 9 (last quad has 1 tile)
                for q in range(n_quads):
                    rr = range(4) if q < 8 else range(1)
                    # Lambda staging is bf16 (it lands in bf16 LT4 anyway) so
                    # the 500 PE transposes use single-pass bf16 weight loads;
                    # fvar/idio tiles (jj 31/32) also stage fp32 rows.
                    raw_bf = rawp.tile([128, 512], BF, tag="rawbf")
                    raw_t = (rawp.tile([128, 512], FP, tag="raw",
                                       name=f"raw{q}")
                             if q >= 7 else None)
                    fcb_q = rawp.tile([1, 2048], BF, tag="fcb")
                    fw = min(2048 * (q + 1), FH) - 2048 * q
                    nc.sync.dma_start(
                        fcb_q[:, 0:fw], fcbRow_d[:, 2048 * q:2048 * q + fw])
                    for r in rr:
                        jj = 4 * q + r
                        if jj < NPREF:
                            fcw_t = fcw_pref[:, :, jj * 512:(jj + 1) * 512]
                        else:
                            fcw_t = fcwp.tile([128, 4, 512], BF, tag="fcw")
                            nc.sync.dma_start(
                                fcw_t,
                                fcwT_d[:, jj * 512:(jj + 1) * 512]
                                .rearrange("(ko p) n -> p ko n", p=128),
                            )
                        pfc = pfcp.tile([128, 512], FP, tag="pfc")
                        mm(pfc[32 * r:32 * (r + 1), :], ones_sb[0:1, :],
                           fcb_q[0:1, 512 * r:512 * (r + 1)],
                           tp=(0, 32 * r), start=True, stop=False)
                        for k in range(4):
                            mm(pfc[32 * r:32 * (r + 1), :],
                               hlast[:, 32 * k:32 * (k + 1)],
                               fcw_t[:, k, :],
                               tp=(0, 32 * r),
                               start=False, stop=(k == 3))
                        s = slice(32 * r, 32 * (r + 1))
                        if jj < 31:
                            if r % 2 == 0:
                                nc.scalar.copy(raw_bf[s, :], pfc[s, :])
                            else:
                                nc.vector.tensor_copy(raw_bf[s, :], pfc[s, :])
                        elif jj == 31:
                            # fp32 rows for fvar+idio, bf16 Lambda cols 0:128
                            nc.scalar.copy(raw_t[s, :], pfc[s, :])
                            nc.vector.tensor_copy(raw_bf[s, 0:128],
                                                  pfc[s, 0:128])
                        else:
                            nc.scalar.copy(raw_t[s, :], pfc[s, :])

                    # Lambda blocks -> per-n-tile transposes into one PSUM
                    # bank -> single cast into LT4 rows 0:32.  NOTE: the
                    # transposes must stay grouped per row-group; interleaving
                    # them across n-tiles corrupts on HW (fp32 transpose
                    # weight loads are two-pass LOW/HIGH).
                    for r in rr:
                        jj = 4 * q + r
                        nblk = 16 if jj < 31 else (4 if jj == 31 else 0)
                        if nblk:
                            pt = ppp.tile([32, 512], BF, tag="pp",
                                          name=f"pt{jj}")
                            for blk in range(nblk):
                                tr(pt[:, 32 * blk:32 * (blk + 1)],
                                   raw_bf[32 * r:32 * (r + 1),
                                          32 * blk:32 * (blk + 1)],
                                   identb_sb[32 * r:32 * (r + 1), :],
                                   (32 * r, 0))
                            a0 = 16 * jj
                            if r % 2 == 0:
                                nc.scalar.copy(
                                    LT4[0:32, :, a0:a0 + nblk],
                                    pt[:, 0:32 * nblk]
                                    .rearrange("f (a b) -> f b a", a=nblk))
                            else:
                                nc.vector.tensor_copy(
                                    LT4[0:32, :, a0:a0 + nblk],
                                    pt[:, 0:32 * nblk]
                                    .rearrange("f (a b) -> f b a", a=nblk))
                        if jj == 31:
                            # fvar: features 16000:16032 = cols 128:160;
                            # transpose once, exp into F4 rows 0:32, then
                            # DMA-replicate to partition offsets 32/64/96
                            ptf = ppp.tile([32, 512], FP, tag="pp",
                                           name="ptf")
                            tr(ptf[:, 0:32], raw_t[96:128, 128:160],
                               identt_sb[96:128, :], (96, 0))
                            nc.scalar.activation(F4[0:32, :], ptf[:, 0:32],
                                                 AF.Exp)
                            for c in range(1, 4):
                                nc.sync.dma_start(
                                    F4[32 * c:32 * (c + 1), :], F4[0:32, :])
                            # idio part 1: features 16032:16384 = cols 160:512
                            nc.sync.dma_start(idio_d[:, 0:352],
                                              raw_t[96:128, 160:512])
                        if jj == 32:
                            # idio part 2: features 16384:16532 = cols 0:148
                            nc.sync.dma_start(idio_d[:, 352:500],
                                              raw_t[0:32, 0:148])

                    # replicate this quad's LT columns to offsets 32/64/96
                    a0, a1 = 16 * 4 * q, min(16 * 4 * (q + 1), NA)
                    if a1 > a0:
                        for c in range(1, 4):
                            nc.sync.dma_start(
                                LT4[32 * c:32 * (c + 1), :, a0:a1],
                                LT4[0:32, :, a0:a1])

                # Sigma per sample: 4 concurrent row-tiled matmuls, evacs
                # into one staging tile, two batched DMAs alternating between
                # the two HWDGE queues (descriptor issue is the bottleneck)
                for b in range(BL):
                    gt4 = gtp.tile([128, 512], BF, tag="gt4")
                    nc.vector.tensor_scalar_mul(gt4[:, 0:500], LT4[:, b, 0:500],
                                                F4[:, b:b + 1])
                    stg = sigw.tile([128, 4, 512], BF, tag="sigstage")
                    for mt in range(4):
                        rows = 128 if mt < 3 else 116
                        ps = ppp.tile([128, 512], FP, tag="pp",
                                      name=f"ps{b}_{mt}")
                        mm(ps[:rows, 0:500],
                           gt4[32 * mt:32 * (mt + 1), 128 * mt:128 * mt + rows],
                           LT4[32 * mt:32 * (mt + 1), b, 0:500],
                           tp=(32 * mt, 0), start=True, stop=True)
                        if mt % 2 == 0:
                            nc.scalar.copy(stg[:rows, mt, 0:500],
                                           ps[:rows, 0:500])
                        else:
                            nc.vector.tensor_copy(stg[:rows, mt, 0:500],
                                                  ps[:rows, 0:500])
                    eng = nc.sync if b % 2 == 0 else nc.scalar
                    eng.dma_start(
                        sigma_d[b, 0:384, :].rearrange("(m p) a -> p m a",
                                                       p=128),
                        stg[:, 0:3, 0:500])
                    eng.dma_start(sigma_d[b, 384:500, :], stg[0:116, 3, 0:500])

    nc.compile()
    return nc


# ---------------------------------------------------------------- entry point

def postprocess(results, inputs):
    idx = np.arange(NA)
    out = np.empty((B_FULL, NA, NA), np.float32)
    for core in range(NCORES):
        sigma = np.asarray(results[core]["sigma"]).astype(np.float32)
        idio = np.exp(np.asarray(results[core]["idio_raw"], np.float32))
        sigma[:, idx, idx] += idio
        out[core * BL:(core + 1) * BL] = sigma
    return out


def kernel(**inputs):
    from concourse.bass_utils import run_bass_kernel_spmd

    prep = host_prep_shared(inputs)
    x = np.asarray(inputs["x"], np.float32)
    in_maps = []
    for core in range(NCORES):
        m = dict(prep)
        m["xT"] = host_prep_x(x[core * BL:(core + 1) * BL])
        in_maps.append(m)

    nc = build_nc()
    res = run_bass_kernel_spmd(nc, in_maps, list(range(NCORES)))
    return postprocess(res.results, inputs)


# revision 34
# speedup vs baseline: 1.0269x; 1.0269x over previous
"""Trainium2 Bass kernel for nn_FactorCovModel.

Model: 2-layer LSTM (H=512) over [B=256, T=64, D=500], last hidden ->
FC [512 -> 16532] -> Sigma = Lambda diag(exp(fv)) Lambda^T + diag(exp(idio)),
output [256, 500, 500].

Sharding: pure data parallel over batch, 32 samples/core on 8 cores.

Per-core design (matmul operands bf16, fp32 PSUM accumulation):
  - Gate axis host-permuted to [i, f, o, g] x hidden-group so PSUM col
    group hg holds hidden slice hg of all four gates; sigmoid covers one
    contiguous [128, 384] op, tanh one [128, 128] op.
  - LSTM gates col-tiled: stationary = x/hT chunk [128, 32] at positions
    (0, 32j); 4 hidden-group strips run concurrently, one PSUM bank each.
  - l1 bias injected via a K=1 ones-row matmul (start of the accum group)
    instead of DVE adds during evac.
  - Emission is software-pipelined: per t we emit l1-MMs(t-1), l0-MMs(t),
    l1-elementwise(t-1), l0-elementwise(t) so the PE never starves while
    ACT/DVE run the nonlinearity of the other layer.
  - Evacs split 2-on-ACT / 2-on-DVE; elementwise kept bf16 where the c
    accumulator doesn't need fp32.
  - FC bias also injected via K=1 matmul; Lambda blocks PE-transposed per
    n-tile into one PSUM bank -> one [32, 512] cast into LT; LT then
    DMA-replicated to partition offsets 32/64/96 so per-sample Sigma runs
    4 concurrent row-tiled matmuls (one per 128-row m-tile).
  - Sigma written to DRAM as bf16 (halves the 32 MB/core output traffic);
    host converts to fp32, mirrors nothing (full square written), applies
    exp to idio raw rows and adds the diagonal.
"""

import os
import sys

sys.path.insert(0, "/opt/trn_rl_repo")

import numpy as np

import concourse.bass as bass
import concourse.mybir as mybir
from concourse import bacc
from concourse.tile import TileContext

FP = mybir.dt.float32
BF = mybir.dt.bfloat16
AF = mybir.ActivationFunctionType

B_FULL, T_FULL, D_IN, H = 256, 64, 500, 512
NCORES = 8
BL = B_FULL // NCORES            # 32 samples per core
NA, NF = 500, 32                 # assets, factors
OUT_DIM = NA * NF + NF + NA      # 16532
NTILE = 512                      # FC feature tile
N_FTILES = 33                    # ceil(16532/512) -> features padded to 16896
FH = N_FTILES * NTILE            # 16896
XCHUNK = 16                      # time steps per streamed xT chunk
NPREF = 24                       # fcw tiles prefetched during the LSTM

# gate-axis permutation: new col (hg, gate', hl) = 512*hg + 128*gate' + hl maps
# to old row og*512 + 128*hg + hl with og = [i,f,o,g] -> torch [i,f,g,o] index.
# With this layout PSUM col group hg holds [i|f|o|g] x 128 lanes of hidden
# slice hg, so sigmoid is one [*, 0:384] op and tanh one [*, 384:512] op.
_OG = [0, 1, 3, 2]
PERM = np.array([_OG[g] * 512 + 128 * hg + hl
                 for hg in range(4) for g in range(4) for hl in range(128)])


# ---------------------------------------------------------------- host prep

def host_prep_shared(inputs):
    import ml_dtypes
    tobf = lambda a: np.ascontiguousarray(a, dtype=ml_dtypes.bfloat16)

    w_ih0 = np.asarray(inputs["w_ih0"])[PERM]
    w_hh0 = np.asarray(inputs["w_hh0"])[PERM]
    b0 = (np.asarray(inputs["b_ih0"]) + np.asarray(inputs["b_hh0"]))[PERM]
    w_ih1 = np.asarray(inputs["w_ih1"])[PERM]
    w_hh1 = np.asarray(inputs["w_hh1"])[PERM]
    b1 = (np.asarray(inputs["b_ih1"]) + np.asarray(inputs["b_hh1"]))[PERM]
    fc_w = np.asarray(inputs["fc_w"])
    fc_b = np.asarray(inputs["fc_b"])

    w0T = np.zeros((512, 2048), np.float32)
    w0T[:500] = w_ih0.T
    w0T[500] = b0
    wh0T = np.ascontiguousarray(w_hh0.T, dtype=np.float32)
    w1T = np.ascontiguousarray(np.concatenate([w_ih1.T, w_hh1.T]), dtype=np.float32)
    b1row = b1.reshape(1, 2048)
    fcwT = np.zeros((512, FH), np.float32)
    fcwT[:, :OUT_DIM] = fc_w.T
    fcbRow = np.zeros((1, FH), np.float32)
    fcbRow[0, :OUT_DIM] = fc_b
    onesT = np.ones((1, 32), np.float32)
    ident = np.ascontiguousarray(np.tile(np.eye(32, dtype=np.float32), (4, 1)))
    identb = ident.copy()
    return dict(w0T=tobf(w0T), wh0T=tobf(wh0T), w1T=tobf(w1T),
                b1row=tobf(b1row), fcwT=tobf(fcwT), fcbRow=tobf(fcbRow),
                onesT=tobf(onesT), identt=np.ascontiguousarray(ident),
                identb=tobf(identb))


def host_prep_x(x_core):
    """x_core [BL, T, 500] -> xT [512, T*BL], (t, b) free order, ones bias row."""
    T = x_core.shape[1]
    import ml_dtypes
    xT = np.zeros((512, T * BL), np.float32)
    xT[:500] = np.asarray(x_core, np.float32).transpose(2, 1, 0).reshape(500, T * BL)
    xT[500] = 1.0
    return np.ascontiguousarray(xT, dtype=ml_dtypes.bfloat16)


# ---------------------------------------------------------------- bass build

def build_nc(T=T_FULL):
    nc = bacc.Bacc("TRN2")

    xT_d = nc.dram_tensor("xT", [512, T * BL], BF, kind="ExternalInput")
    w0T_d = nc.dram_tensor("w0T", [512, 2048], BF, kind="ExternalInput")
    wh0T_d = nc.dram_tensor("wh0T", [512, 2048], BF, kind="ExternalInput")
    w1T_d = nc.dram_tensor("w1T", [1024, 2048], BF, kind="ExternalInput")
    b1row_d = nc.dram_tensor("b1row", [1, 2048], BF, kind="ExternalInput")
    fcwT_d = nc.dram_tensor("fcwT", [512, FH], BF, kind="ExternalInput")
    fcbRow_d = nc.dram_tensor("fcbRow", [1, FH], BF, kind="ExternalInput")
    onesT_d = nc.dram_tensor("onesT", [1, 32], BF, kind="ExternalInput")
    identt_d = nc.dram_tensor("identt", [128, 32], FP, kind="ExternalInput")
    identb_d = nc.dram_tensor("identb", [128, 32], BF, kind="ExternalInput")

    sigma_d = nc.dram_tensor("sigma", [BL, NA, NA], BF, kind="ExternalOutput")
    idio_d = nc.dram_tensor("idio_raw", [BL, NA], FP, kind="ExternalOutput")

    def mm(out, lhsT, rhs, tp, **kw):
        nc.tensor.matmul(out, lhsT, rhs,
                         tile_position=tp, skip_group_check=True, **kw)

    def tr(out, in_, identity, tp):
        nc.tensor.matmul(out, in_, identity, is_transpose=True,
                         tile_position=tp, skip_group_check=True)

    with TileContext(nc) as tc:
        with tc.tile_pool(name="persist", bufs=1) as persist:
            b1row_sb = persist.tile([1, 2048], BF)
            nc.sync.dma_start(b1row_sb, b1row_d[:, :])
            ones_sb = persist.tile([1, 32], BF)
            nc.sync.dma_start(ones_sb, onesT_d[:, :])
            identt_sb = persist.tile([128, 32], FP)
            nc.sync.dma_start(identt_sb, identt_d[:, :])
            identb_sb = persist.tile([128, 32], BF)
            nc.sync.dma_start(identb_sb, identb_d[:, :])
            hlast = persist.tile([128, 128], BF)  # final h1T, chunk-major cols
            # fcw prefetch tile: loaded AFTER the LSTM weights are issued so
            # the 12 MB transfer doesn't head-of-line block the first step
            fcw_pref = persist.tile([128, 4, NPREF * 512], BF)

            # ---------------- phase 1: LSTM ----------------
            with (
                tc.tile_pool(name="wconst", bufs=1) as wconst,
                tc.tile_pool(name="xring", bufs=2) as xring,
                tc.tile_pool(name="state", bufs=2) as state,
                tc.tile_pool(name="work", bufs=2) as work,
                tc.tile_pool(name="p0", bufs=4, space="PSUM") as p0,
                tc.tile_pool(name="p1", bufs=4, space="PSUM") as p1,
            ):
                w0T_sb = wconst.tile([128, 4, 2048], BF)
                wh0T_sb = wconst.tile([128, 4, 2048], BF)
                w1T_sb = wconst.tile([128, 8, 2048], BF)

                xch = min(XCHUNK, T)
                n_xchunks = (T + xch - 1) // xch
                x_tiles = {}

                def load_xchunk(ci):
                    if ci >= n_xchunks or ci in x_tiles:
                        return
                    xt = xring.tile([128, 4, xch * BL], BF, tag="xchunk")
                    nc.sync.dma_start(
                        xt,
                        xT_d[:, ci * xch * BL:(ci + 1) * xch * BL]
                        .rearrange("(ko p) tb -> p ko tb", p=128),
                    )
                    x_tiles[ci] = xt

                # x chunk 0 first (smallest critical load), then weights in
                # consumption order, then the bulky fcw prefetch
                load_xchunk(0)
                nc.sync.dma_start(w0T_sb, w0T_d.rearrange("(ko p) g -> p ko g", p=128))
                nc.sync.dma_start(wh0T_sb, wh0T_d.rearrange("(ko p) g -> p ko g", p=128))
                nc.sync.dma_start(w1T_sb, w1T_d.rearrange("(ko p) g -> p ko g", p=128))
                load_xchunk(1)
                nc.sync.dma_start(
                    fcw_pref,
                    fcwT_d[:, 0:NPREF * 512]
                    .rearrange("(ko p) n -> p ko n", p=128),
                )

                def mm_xproj(t):
                    """x-projection of step t into fresh l0 gate banks."""
                    ci, tl = t // xch, t % xch
                    if tl == 0:
                        load_xchunk(ci + 1)
                    xt = x_tiles[ci]
                    pgs = [p0.tile([128, 512], FP, tag="g", name=f"g0_{t}_{j}")
                           for j in range(4)]
                    for k in range(4):
                        lhsT = xt[:, k, tl * BL:(tl + 1) * BL]
                        for j in range(4):
                            mm(pgs[j][32 * j:32 * (j + 1), :], lhsT,
                               w0T_sb[:, k, 512 * j:512 * (j + 1)],
                               tp=(0, 32 * j),
                               start=(k == 0), stop=(t == 0 and k == 3))
                    return pgs

                def mm_rec(t, pgs):
                    """recurrent accumulation of layer 0, step t."""
                    h0T = st["h0T"]
                    for k in range(4):
                        lhsT = h0T[:, 32 * k:32 * (k + 1)]
                        for j in range(4):
                            mm(pgs[j][32 * j:32 * (j + 1), :], lhsT,
                               wh0T_sb[:, k, 512 * j:512 * (j + 1)],
                               tp=(0, 32 * j),
                               start=False, stop=(k == 3))

                def mm_l1a(t):
                    """bias + h0T(t) half of layer 1's accumulation."""
                    pgs = [p1.tile([128, 512], FP, tag="g", name=f"g1_{t}_{j}")
                           for j in range(4)]
                    for j in range(4):
                        mm(pgs[j][32 * j:32 * (j + 1), :], ones_sb[0:1, :],
                           b1row_sb[0:1, 512 * j:512 * (j + 1)],
                           tp=(0, 32 * j), start=True, stop=False)
                    h0T = st["h0T"]
                    for k in range(4):
                        lhsT = h0T[:, 32 * k:32 * (k + 1)]
                        for j in range(4):
                            mm(pgs[j][32 * j:32 * (j + 1), :], lhsT,
                               w1T_sb[:, k, 512 * j:512 * (j + 1)],
                               tp=(0, 32 * j),
                               start=False, stop=(t == 0 and k == 3))
                    return pgs

                def mm_l1b(t, pgs):
                    """h1T(t-1) half of layer 1's accumulation (t >= 1)."""
                    h1T = st["h1T"]
                    for k in range(4):
                        lhsT = h1T[:, 32 * k:32 * (k + 1)]
                        for j in range(4):
                            mm(pgs[j][32 * j:32 * (j + 1), :], lhsT,
                               w1T_sb[:, 4 + k, 512 * j:512 * (j + 1)],
                               tp=(0, 32 * j),
                               start=False, stop=(k == 3))

                st = {"h0T": None, "h1T": None, "c0": None, "c1": None}

                def elem_nl(t, layer, pgs):
                    """evac + nonlinearity + DMA-xbar transpose -> hT."""
                    tag = f"l{layer}"
                    a = work.tile([128, 512], FP, tag=f"a_{tag}")
                    for j in range(4):
                        s = slice(32 * j, 32 * (j + 1))
                        if j % 2 == 0:
                            nc.scalar.copy(a[s, :], pgs[j][s, :])
                        else:
                            nc.vector.tensor_copy(a[s, :], pgs[j][s, :])
                    act = work.tile([128, 512], FP, tag=f"act_{tag}")
                    nc.scalar.activation(act[:, 0:384], a[:, 0:384], AF.Sigmoid)
                    nc.scalar.activation(act[:, 384:512], a[:, 384:512], AF.Tanh)
                    t1 = work.tile([128, 128], FP, tag=f"t1_{tag}")
                    nc.vector.tensor_mul(t1, act[:, 0:128], act[:, 384:512])
                    cprev = st[f"c{layer}"]
                    if cprev is None:
                        cn = t1
                    else:
                        t2 = work.tile([128, 128], FP, tag=f"t2_{tag}")
                        nc.vector.tensor_mul(t2, act[:, 128:256], cprev)
                        cn = work.tile([128, 128], FP, tag=f"c_{tag}")
                        nc.vector.tensor_add(cn, t1, t2)
                    st[f"c{layer}"] = cn
                    tcn = work.tile([128, 128], FP, tag=f"tc_{tag}")
                    nc.scalar.activation(tcn, cn, AF.Tanh)
                    hh = work.tile([128, 128], BF, tag=f"h_{tag}")
                    nc.vector.tensor_mul(hh, act[:, 256:384], tcn)
                    ht = state.tile([128, 128], BF, tag=f"ht_{tag}")
                    nc.sync.dma_start(ht, hh, transpose=True)
                    st[f"h{layer}T"] = ht

                def warmers(n):
                    # standalone LDWEIGHTS keep the PE's HAM activity monitor
                    # busy through the nonlinearity window (no PSUM side
                    # effects; every matmul reloads its own stationary)
                    for _ in range(n):
                        nc.tensor.ldweights(w0T_sb[:, 0, 0:128])

                # Software-pipelined emission; PE FIFO per step:
                #   rec(t) | l1-bias+h0T(t-1) | xproj(t+1) | l1-h1T(t-1) |
                #   warmers  -- while ACT/DVE run l0(t) then l1(t-1)
                # elementwise and the h transposes go through the DMA xbar.
                pgs0 = mm_xproj(0)
                pgs1 = None
                for t in range(T):
                    if t > 0:
                        mm_rec(t, pgs0)
                        pgs1 = mm_l1a(t - 1)
                    h0args = (t, 0, pgs0)
                    elem_nl(*h0args)
                    if t + 1 < T:
                        pgs0 = mm_xproj(t + 1)
                    if t > 1:
                        mm_l1b(t - 1, pgs1)
                    warmers(12)
                    if t > 0:
                        elem_nl(t - 1, 1, pgs1)
                pgs1 = mm_l1a(T - 1)
                mm_l1b(T - 1, pgs1)
                elem_nl(T - 1, 1, pgs1)
                nc.vector.tensor_copy(hlast, st["h1T"])

            # ---------------- phase 2: FC + Lambda layout + Sigma ----------------
            with (
                tc.tile_pool(name="fcw", bufs=3) as fcwp,
                tc.tile_pool(name="rawp", bufs=3) as rawp,
                tc.tile_pool(name="lt", bufs=1) as ltp,
                tc.tile_pool(name="gt", bufs=3) as gtp,
                tc.tile_pool(name="sigw", bufs=6) as sigw,
                tc.tile_pool(name="pfc", bufs=4, space="PSUM") as pfcp,
                tc.tile_pool(name="pp", bufs=4, space="PSUM") as ppp,
            ):
                # LT4: [128 = (rep q, factor f), 32 b, 500 assets] bf16
                LT4 = ltp.tile([128, 32, 512], BF)
                F4 = ltp.tile([128, 32], FP)   # exp(fvar raw + bias), 4 reps

                n_quads = (N_FTILES + 3) // 4          # 9 (last quad has 1 tile)

                def emit_lambda(q, rr, raw_t):
                    # Lambda blocks -> per-n-tile transposes into one PSUM
                    # bank -> single cast into LT4 rows 0:32
                    for r in rr:
                        jj = 4 * q + r
                        nblk = 16 if jj < 31 else (4 if jj == 31 else 0)
                        if nblk:
                            pt = ppp.tile([32, 512], FP, tag="pp",
                                          name=f"pt{jj}")
                            for blk in range(nblk):
                                tr(pt[:, 32 * blk:32 * (blk + 1)],
                                   raw_t[32 * r:32 * (r + 1),
                                         32 * blk:32 * (blk + 1)],
                                   identt_sb[32 * r:32 * (r + 1), :],
                                   (32 * r, 0))
                            a0 = 16 * jj
                            if r % 2 == 0:
                                nc.scalar.copy(
                                    LT4[0:32, :, a0:a0 + nblk],
                                    pt[:, 0:32 * nblk]
                                    .rearrange("f (a b) -> f b a", a=nblk))
                            else:
                                nc.vector.tensor_copy(
                                    LT4[0:32, :, a0:a0 + nblk],
                                    pt[:, 0:32 * nblk]
                                    .rearrange("f (a b) -> f b a", a=nblk))
                        if jj == 31:
                            # fvar: features 16000:16032 = cols 128:160;
                            # transpose once, exp into F4 rows 0:32, then
                            # DMA-replicate to partition offsets 32/64/96
                            ptf = ppp.tile([32, 512], FP, tag="pp",
                                           name="ptf")
                            tr(ptf[:, 0:32], raw_t[96:128, 128:160],
                               identt_sb[96:128, :], (96, 0))
                            nc.scalar.activation(F4[0:32, :], ptf[:, 0:32],
                                                 AF.Exp)
                            for c in range(1, 4):
                                nc.sync.dma_start(
                                    F4[32 * c:32 * (c + 1), :], F4[0:32, :])
                            # idio part 1: features 16032:16384 = cols 160:512
                            nc.sync.dma_start(idio_d[:, 0:352],
                                              raw_t[96:128, 160:512])
                        if jj == 32:
                            # idio part 2: features 16384:16532 = cols 0:148
                            nc.sync.dma_start(idio_d[:, 352:500],
                                              raw_t[0:32, 0:148])

                    # replicate this quad's LT columns to offsets 32/64/96
                    a0, a1 = 16 * 4 * q, min(16 * 4 * (q + 1), NA)
                    if a1 > a0:
                        for c in range(1, 4):
                            nc.sync.dma_start(
                                LT4[32 * c:32 * (c + 1), :, a0:a1],
                                LT4[0:32, :, a0:a1])

                # Software-pipelined quad loop: quad q's FC matmuls + evacs
                # are emitted before quad q-1's transposes/casts so the PE
                # never FIFO-blocks on the previous quad's evac, and the
                # ACT/DVE casts overlap the next quad's matmuls.
                pend = None
                for q in range(n_quads):
                    rr = range(4) if q < 8 else range(1)
                    raw_t = rawp.tile([128, 512], FP, tag="raw",
                                      name=f"raw{q}")
                    fcb_q = rawp.tile([1, 2048], BF, tag="fcb",
                                      name=f"fcb{q}")
                    fw = min(2048 * (q + 1), FH) - 2048 * q
                    nc.sync.dma_start(
                        fcb_q[:, 0:fw], fcbRow_d[:, 2048 * q:2048 * q + fw])
                    for r in rr:
                        jj = 4 * q + r
                        if jj < NPREF:
                            fcw_t = fcw_pref[:, :, jj * 512:(jj + 1) * 512]
                        else:
                            fcw_t = fcwp.tile([128, 4, 512], BF, tag="fcw",
                                              name=f"fcw{jj}")
                            nc.sync.dma_start(
                                fcw_t,
                                fcwT_d[:, jj * 512:(jj + 1) * 512]
                                .rearrange("(ko p) n -> p ko n", p=128),
                            )
                        pfc = pfcp.tile([128, 512], FP, tag="pfc",
                                        name=f"pfc{jj}")
                        mm(pfc[32 * r:32 * (r + 1), :], ones_sb[0:1, :],
                           fcb_q[0:1, 512 * r:512 * (r + 1)],
                           tp=(0, 32 * r), start=True, stop=False)
                        for k in range(4):
                            mm(pfc[32 * r:32 * (r + 1), :],
                               hlast[:, 32 * k:32 * (k + 1)],
                               fcw_t[:, k, :],
                               tp=(0, 32 * r),
                               start=False, stop=(k == 3))
                        s = slice(32 * r, 32 * (r + 1))
                        if r % 2 == 0:
                            nc.scalar.copy(raw_t[s, :], pfc[s, :])
                        else:
                            nc.vector.tensor_copy(raw_t[s, :], pfc[s, :])
                    if pend is not None:
                        emit_lambda(*pend)
                    pend = (q, rr, raw_t)
                emit_lambda(*pend)

                # Sigma per sample: 4 concurrent row-tiled matmuls, evacs
                # into one staging tile, two batched DMAs alternating between
                # the two HWDGE queues (descriptor issue is the bottleneck)
                for b in range(BL):
                    gt4 = gtp.tile([128, 512], BF, tag="gt4")
                    nc.vector.tensor_scalar_mul(gt4[:, 0:500], LT4[:, b, 0:500],
                                                F4[:, b:b + 1])
                    stg = sigw.tile([128, 4, 512], BF, tag="sigstage")
                    for mt in range(4):
                        rows = 128 if mt < 3 else 116
                        ps = ppp.tile([128, 512], FP, tag="pp",
                                      name=f"ps{b}_{mt}")
                        mm(ps[:rows, 0:500],
                           gt4[32 * mt:32 * (mt + 1), 128 * mt:128 * mt + rows],
                           LT4[32 * mt:32 * (mt + 1), b, 0:500],
                           tp=(32 * mt, 0), start=True, stop=True)
                        if mt % 2 == 0:
                            nc.scalar.copy(stg[:rows, mt, 0:500],
                                           ps[:rows, 0:500])
                        else:
                            nc.vector.tensor_copy(stg[:rows, mt, 0:500],
                                                  ps[:rows, 0:500])
                    eng = nc.sync if b % 2 == 0 else nc.scalar
                    eng.dma_start(
                        sigma_d[b, 0:384, :].rearrange("(m p) a -> p m a",
                                                       p=128),
                        stg[:, 0:3, 0:500])
                    eng.dma_start(sigma_d[b, 384:500, :], stg[0:116, 3, 0:500])

    nc.compile()
    return nc


# ---------------------------------------------------------------- entry point

def postprocess(results, inputs):
    idx = np.arange(NA)
    out = np.empty((B_FULL, NA, NA), np.float32)
    for core in range(NCORES):
        sigma = np.asarray(results[core]["sigma"]).astype(np.float32)
        idio = np.exp(np.asarray(results[core]["idio_raw"], np.float32))
        sigma[:, idx, idx] += idio
        out[core * BL:(core + 1) * BL] = sigma
    return out


def kernel(**inputs):
    from concourse.bass_utils import run_bass_kernel_spmd

    prep = host_prep_shared(inputs)
    x = np.asarray(inputs["x"], np.float32)
    in_maps = []
    for core in range(NCORES):
        m = dict(prep)
        m["xT"] = host_prep_x(x[core * BL:(core + 1) * BL])
        in_maps.append(m)

    nc = build_nc()
    res = run_bass_kernel_spmd(nc, in_maps, list(range(NCORES)))
    return postprocess(res.results, inputs)


# revision 37
# speedup vs baseline: 1.0560x; 1.0283x over previous
"""Trainium2 Bass kernel for nn_FactorCovModel.

Model: 2-layer LSTM (H=512) over [B=256, T=64, D=500], last hidden ->
FC [512 -> 16532] -> Sigma = Lambda diag(exp(fv)) Lambda^T + diag(exp(idio)),
output [256, 500, 500].

Sharding: pure data parallel over batch, 32 samples/core on 8 cores.

Per-core design (matmul operands bf16, fp32 PSUM accumulation):
  - Gate axis host-permuted to [i, f, o, g] x hidden-group so PSUM col
    group hg holds hidden slice hg of all four gates; sigmoid covers one
    contiguous [128, 384] op, tanh one [128, 128] op.
  - LSTM gates col-tiled: stationary = x/hT chunk [128, 32] at positions
    (0, 32j); 4 hidden-group strips run concurrently, one PSUM bank each.
  - l1 bias injected via a K=1 ones-row matmul (start of the accum group)
    instead of DVE adds during evac.
  - Emission is software-pipelined: per t we emit l1-MMs(t-1), l0-MMs(t),
    l1-elementwise(t-1), l0-elementwise(t) so the PE never starves while
    ACT/DVE run the nonlinearity of the other layer.
  - Evacs split 2-on-ACT / 2-on-DVE; elementwise kept bf16 where the c
    accumulator doesn't need fp32.
  - FC bias also injected via K=1 matmul; Lambda blocks PE-transposed per
    n-tile into one PSUM bank -> one [32, 512] cast into LT; LT then
    DMA-replicated to partition offsets 32/64/96 so per-sample Sigma runs
    4 concurrent row-tiled matmuls (one per 128-row m-tile).
  - Sigma written to DRAM as bf16 (halves the 32 MB/core output traffic);
    host converts to fp32, mirrors nothing (full square written), applies
    exp to idio raw rows and adds the diagonal.
"""

import os
import sys

sys.path.insert(0, "/opt/trn_rl_repo")

import numpy as np

import concourse.bass as bass
import concourse.mybir as mybir
from concourse import bacc
from concourse.tile import TileContext

FP = mybir.dt.float32
BF = mybir.dt.bfloat16
AF = mybir.ActivationFunctionType

B_FULL, T_FULL, D_IN, H = 256, 64, 500, 512
NCORES = 8
BL = B_FULL // NCORES            # 32 samples per core
NA, NF = 500, 32                 # assets, factors
OUT_DIM = NA * NF + NF + NA      # 16532
NTILE = 512                      # FC feature tile
N_FTILES = 33                    # ceil(16532/512) -> features padded to 16896
FH = N_FTILES * NTILE            # 16896
XCHUNK = 16                      # time steps per streamed xT chunk
NPREF = 24                       # fcw tiles prefetched during the LSTM

# gate-axis permutation: new col (hg, gate', hl) = 512*hg + 128*gate' + hl maps
# to old row og*512 + 128*hg + hl with og = [i,f,o,g] -> torch [i,f,g,o] index.
# With this layout PSUM col group hg holds [i|f|o|g] x 128 lanes of hidden
# slice hg, so sigmoid is one [*, 0:384] op and tanh one [*, 384:512] op.
_OG = [0, 1, 3, 2]
PERM = np.array([_OG[g] * 512 + 128 * hg + hl
                 for hg in range(4) for g in range(4) for hl in range(128)])


# ---------------------------------------------------------------- host prep

def host_prep_shared(inputs):
    import ml_dtypes
    tobf = lambda a: np.ascontiguousarray(a, dtype=ml_dtypes.bfloat16)

    w_ih0 = np.asarray(inputs["w_ih0"])[PERM]
    w_hh0 = np.asarray(inputs["w_hh0"])[PERM]
    b0 = (np.asarray(inputs["b_ih0"]) + np.asarray(inputs["b_hh0"]))[PERM]
    w_ih1 = np.asarray(inputs["w_ih1"])[PERM]
    w_hh1 = np.asarray(inputs["w_hh1"])[PERM]
    b1 = (np.asarray(inputs["b_ih1"]) + np.asarray(inputs["b_hh1"]))[PERM]
    fc_w = np.asarray(inputs["fc_w"])
    fc_b = np.asarray(inputs["fc_b"])

    w0T = np.zeros((512, 2048), np.float32)
    w0T[:500] = w_ih0.T
    w0T[500] = b0
    wh0T = np.ascontiguousarray(w_hh0.T, dtype=np.float32)
    w1T = np.ascontiguousarray(np.concatenate([w_ih1.T, w_hh1.T]), dtype=np.float32)
    b1row = b1.reshape(1, 2048)
    fcwT = np.zeros((512, FH), np.float32)
    fcwT[:, :OUT_DIM] = fc_w.T
    fcbRow = np.zeros((1, FH), np.float32)
    fcbRow[0, :OUT_DIM] = fc_b
    onesT = np.ones((1, 32), np.float32)
    ident = np.ascontiguousarray(np.tile(np.eye(32, dtype=np.float32), (4, 1)))
    identb = ident.copy()
    return dict(w0T=tobf(w0T), wh0T=tobf(wh0T), w1T=tobf(w1T),
                b1row=tobf(b1row), fcwT=tobf(fcwT), fcbRow=tobf(fcbRow),
                onesT=tobf(onesT), identt=np.ascontiguousarray(ident),
                identb=tobf(identb))


def host_prep_x(x_core):
    """x_core [BL, T, 500] -> xT [512, T*BL], (t, b) free order, ones bias row."""
    T = x_core.shape[1]
    import ml_dtypes
    xT = np.zeros((512, T * BL), np.float32)
    xT[:500] = np.asarray(x_core, np.float32).transpose(2, 1, 0).reshape(500, T * BL)
    xT[500] = 1.0
    return np.ascontiguousarray(xT, dtype=ml_dtypes.bfloat16)


# ---------------------------------------------------------------- bass build

def build_nc(T=T_FULL):
    nc = bacc.Bacc("TRN2")

    xT_d = nc.dram_tensor("xT", [512, T * BL], BF, kind="ExternalInput")
    w0T_d = nc.dram_tensor("w0T", [512, 2048], BF, kind="ExternalInput")
    wh0T_d = nc.dram_tensor("wh0T", [512, 2048], BF, kind="ExternalInput")
    w1T_d = nc.dram_tensor("w1T", [1024, 2048], BF, kind="ExternalInput")
    b1row_d = nc.dram_tensor("b1row", [1, 2048], BF, kind="ExternalInput")
    fcwT_d = nc.dram_tensor("fcwT", [512, FH], BF, kind="ExternalInput")
    fcbRow_d = nc.dram_tensor("fcbRow", [1, FH], BF, kind="ExternalInput")
    onesT_d = nc.dram_tensor("onesT", [1, 32], BF, kind="ExternalInput")
    identt_d = nc.dram_tensor("identt", [128, 32], FP, kind="ExternalInput")
    identb_d = nc.dram_tensor("identb", [128, 32], BF, kind="ExternalInput")

    # sigma stored in stage-tile layout [b, p, mt, a] (4 KB contiguous per
    # partition -> line-rate DMA); host reorders rows to [b, mt*128+p, a]
    sigma_d = nc.dram_tensor("sigma", [BL, 128, 4, NTILE], BF,
                             kind="ExternalOutput")
    idio_d = nc.dram_tensor("idio_raw", [BL, NA], FP, kind="ExternalOutput")

    def mm(out, lhsT, rhs, tp, **kw):
        nc.tensor.matmul(out, lhsT, rhs,
                         tile_position=tp, skip_group_check=True, **kw)

    def tr(out, in_, identity, tp):
        nc.tensor.matmul(out, in_, identity, is_transpose=True,
                         tile_position=tp, skip_group_check=True)

    with TileContext(nc) as tc:
        with tc.tile_pool(name="persist", bufs=1) as persist:
            b1row_sb = persist.tile([1, 2048], BF)
            nc.sync.dma_start(b1row_sb, b1row_d[:, :])
            ones_sb = persist.tile([1, 32], BF)
            nc.sync.dma_start(ones_sb, onesT_d[:, :])
            identt_sb = persist.tile([128, 32], FP)
            nc.sync.dma_start(identt_sb, identt_d[:, :])
            identb_sb = persist.tile([128, 32], BF)
            nc.sync.dma_start(identb_sb, identb_d[:, :])
            hlast = persist.tile([128, 128], BF)  # final h1T, chunk-major cols
            # fcw prefetch tile: loaded AFTER the LSTM weights are issued so
            # the 12 MB transfer doesn't head-of-line block the first step
            fcw_pref = persist.tile([128, 4, NPREF * 512], BF)

            # ---------------- phase 1: LSTM ----------------
            with (
                tc.tile_pool(name="wconst", bufs=1) as wconst,
                tc.tile_pool(name="xring", bufs=2) as xring,
                tc.tile_pool(name="state", bufs=2) as state,
                tc.tile_pool(name="work", bufs=2) as work,
                tc.tile_pool(name="p0", bufs=4, space="PSUM") as p0,
                tc.tile_pool(name="p1", bufs=4, space="PSUM") as p1,
            ):
                w0T_sb = wconst.tile([128, 4, 2048], BF)
                wh0T_sb = wconst.tile([128, 4, 2048], BF)
                w1T_sb = wconst.tile([128, 8, 2048], BF)

                xch = min(XCHUNK, T)
                n_xchunks = (T + xch - 1) // xch
                x_tiles = {}

                def load_xchunk(ci):
                    if ci >= n_xchunks or ci in x_tiles:
                        return
                    xt = xring.tile([128, 4, xch * BL], BF, tag="xchunk")
                    nc.sync.dma_start(
                        xt,
                        xT_d[:, ci * xch * BL:(ci + 1) * xch * BL]
                        .rearrange("(ko p) tb -> p ko tb", p=128),
                    )
                    x_tiles[ci] = xt

                # x chunk 0 first (smallest critical load), then weights in
                # consumption order, then the bulky fcw prefetch
                load_xchunk(0)
                nc.sync.dma_start(w0T_sb, w0T_d.rearrange("(ko p) g -> p ko g", p=128))
                nc.sync.dma_start(wh0T_sb, wh0T_d.rearrange("(ko p) g -> p ko g", p=128))
                nc.sync.dma_start(w1T_sb, w1T_d.rearrange("(ko p) g -> p ko g", p=128))
                load_xchunk(1)
                nc.sync.dma_start(
                    fcw_pref,
                    fcwT_d[:, 0:NPREF * 512]
                    .rearrange("(ko p) n -> p ko n", p=128),
                )

                def mm_xproj(t):
                    """x-projection of step t into fresh l0 gate banks."""
                    ci, tl = t // xch, t % xch
                    if tl == 0:
                        load_xchunk(ci + 1)
                    xt = x_tiles[ci]
                    pgs = [p0.tile([128, 512], FP, tag="g", name=f"g0_{t}_{j}")
                           for j in range(4)]
                    for k in range(4):
                        lhsT = xt[:, k, tl * BL:(tl + 1) * BL]
                        for j in range(4):
                            mm(pgs[j][32 * j:32 * (j + 1), :], lhsT,
                               w0T_sb[:, k, 512 * j:512 * (j + 1)],
                               tp=(0, 32 * j),
                               start=(k == 0), stop=(t == 0 and k == 3))
                    return pgs

                def mm_rec(t, pgs):
                    """recurrent accumulation of layer 0, step t."""
                    h0T = st["h0T"]
                    for k in range(4):
                        lhsT = h0T[:, 32 * k:32 * (k + 1)]
                        for j in range(4):
                            mm(pgs[j][32 * j:32 * (j + 1), :], lhsT,
                               wh0T_sb[:, k, 512 * j:512 * (j + 1)],
                               tp=(0, 32 * j),
                               start=False, stop=(k == 3))

                def mm_l1a(t):
                    """bias + h0T(t) half of layer 1's accumulation."""
                    pgs = [p1.tile([128, 512], FP, tag="g", name=f"g1_{t}_{j}")
                           for j in range(4)]
                    for j in range(4):
                        mm(pgs[j][32 * j:32 * (j + 1), :], ones_sb[0:1, :],
                           b1row_sb[0:1, 512 * j:512 * (j + 1)],
                           tp=(0, 32 * j), start=True, stop=False)
                    h0T = st["h0T"]
                    for k in range(4):
                        lhsT = h0T[:, 32 * k:32 * (k + 1)]
                        for j in range(4):
                            mm(pgs[j][32 * j:32 * (j + 1), :], lhsT,
                               w1T_sb[:, k, 512 * j:512 * (j + 1)],
                               tp=(0, 32 * j),
                               start=False, stop=(t == 0 and k == 3))
                    return pgs

                def mm_l1b(t, pgs):
                    """h1T(t-1) half of layer 1's accumulation (t >= 1)."""
                    h1T = st["h1T"]
                    for k in range(4):
                        lhsT = h1T[:, 32 * k:32 * (k + 1)]
                        for j in range(4):
                            mm(pgs[j][32 * j:32 * (j + 1), :], lhsT,
                               w1T_sb[:, 4 + k, 512 * j:512 * (j + 1)],
                               tp=(0, 32 * j),
                               start=False, stop=(k == 3))

                st = {"h0T": None, "h1T": None, "c0": None, "c1": None}

                def elem_nl(t, layer, pgs):
                    """evac + nonlinearity + DMA-xbar transpose -> hT."""
                    tag = f"l{layer}"
                    a = work.tile([128, 512], FP, tag=f"a_{tag}")
                    for j in range(4):
                        s = slice(32 * j, 32 * (j + 1))
                        if j % 2 == 0:
                            nc.scalar.copy(a[s, :], pgs[j][s, :])
                        else:
                            nc.vector.tensor_copy(a[s, :], pgs[j][s, :])
                    act = work.tile([128, 512], FP, tag=f"act_{tag}")
                    nc.scalar.activation(act[:, 0:384], a[:, 0:384], AF.Sigmoid)
                    nc.scalar.activation(act[:, 384:512], a[:, 384:512], AF.Tanh)
                    t1 = work.tile([128, 128], FP, tag=f"t1_{tag}")
                    nc.vector.tensor_mul(t1, act[:, 0:128], act[:, 384:512])
                    cprev = st[f"c{layer}"]
                    if cprev is None:
                        cn = t1
                    else:
                        t2 = work.tile([128, 128], FP, tag=f"t2_{tag}")
                        nc.vector.tensor_mul(t2, act[:, 128:256], cprev)
                        cn = work.tile([128, 128], FP, tag=f"c_{tag}")
                        nc.vector.tensor_add(cn, t1, t2)
                    st[f"c{layer}"] = cn
                    tcn = work.tile([128, 128], FP, tag=f"tc_{tag}")
                    nc.scalar.activation(tcn, cn, AF.Tanh)
                    hh = work.tile([128, 128], BF, tag=f"h_{tag}")
                    nc.vector.tensor_mul(hh, act[:, 256:384], tcn)
                    ht = state.tile([128, 128], BF, tag=f"ht_{tag}")
                    nc.sync.dma_start(ht, hh, transpose=True)
                    st[f"h{layer}T"] = ht

                def warmers(n):
                    # standalone LDWEIGHTS keep the PE's HAM activity monitor
                    # busy through the nonlinearity window (no PSUM side
                    # effects; every matmul reloads its own stationary)
                    for _ in range(n):
                        nc.tensor.ldweights(w0T_sb[:, 0, 0:128])

                # Software-pipelined emission; PE FIFO per step:
                #   rec(t) | l1-bias+h0T(t-1) | xproj(t+1) | l1-h1T(t-1) |
                #   warmers  -- while ACT/DVE run l0(t) then l1(t-1)
                # elementwise and the h transposes go through the DMA xbar.
                pgs0 = mm_xproj(0)
                pgs1 = None
                for t in range(T):
                    if t > 0:
                        mm_rec(t, pgs0)
                        pgs1 = mm_l1a(t - 1)
                    h0args = (t, 0, pgs0)
                    elem_nl(*h0args)
                    if t + 1 < T:
                        pgs0 = mm_xproj(t + 1)
                    if t > 1:
                        mm_l1b(t - 1, pgs1)
                    warmers(12)
                    if t > 0:
                        elem_nl(t - 1, 1, pgs1)
                pgs1 = mm_l1a(T - 1)
                mm_l1b(T - 1, pgs1)
                elem_nl(T - 1, 1, pgs1)
                nc.vector.tensor_copy(hlast, st["h1T"])

            # ---------------- phase 2: FC + Lambda layout + Sigma ----------------
            with (
                tc.tile_pool(name="fcw", bufs=3) as fcwp,
                tc.tile_pool(name="rawp", bufs=3) as rawp,
                tc.tile_pool(name="lt", bufs=1) as ltp,
                tc.tile_pool(name="gt", bufs=3) as gtp,
                tc.tile_pool(name="sigw", bufs=6) as sigw,
                tc.tile_pool(name="pfc", bufs=4, space="PSUM") as pfcp,
                tc.tile_pool(name="pp", bufs=4, space="PSUM") as ppp,
            ):
                # LT4: [128 = (rep q, factor f), 32 b, 500 assets] bf16
                LT4 = ltp.tile([128, 32, 512], BF)
                F4 = ltp.tile([128, 32], FP)   # exp(fvar raw + bias), 4 reps

                n_quads = (N_FTILES + 3) // 4          # 9 (last quad has 1 tile)

                def emit_lambda(q, rr, raw_t):
                    # Lambda blocks -> per-n-tile transposes into one PSUM
                    # bank -> single cast into LT4 rows 0:32
                    for r in rr:
                        jj = 4 * q + r
                        nblk = 16 if jj < 31 else (4 if jj == 31 else 0)
                        if nblk:
                            pt = ppp.tile([32, 512], FP, tag="pp",
                                          name=f"pt{jj}")
                            for blk in range(nblk):
                                tr(pt[:, 32 * blk:32 * (blk + 1)],
                                   raw_t[32 * r:32 * (r + 1),
                                         32 * blk:32 * (blk + 1)],
                                   identt_sb[32 * r:32 * (r + 1), :],
                                   (32 * r, 0))
                            a0 = 16 * jj
                            if r % 2 == 0:
                                nc.scalar.copy(
                                    LT4[0:32, :, a0:a0 + nblk],
                                    pt[:, 0:32 * nblk]
                                    .rearrange("f (a b) -> f b a", a=nblk))
                            else:
                                nc.vector.tensor_copy(
                                    LT4[0:32, :, a0:a0 + nblk],
                                    pt[:, 0:32 * nblk]
                                    .rearrange("f (a b) -> f b a", a=nblk))
                        if jj == 31:
                            # fvar: features 16000:16032 = cols 128:160;
                            # transpose once, exp into F4 rows 0:32, then
                            # DMA-replicate to partition offsets 32/64/96
                            ptf = ppp.tile([32, 512], FP, tag="pp",
                                           name="ptf")
                            tr(ptf[:, 0:32], raw_t[96:128, 128:160],
                               identt_sb[96:128, :], (96, 0))
                            nc.scalar.activation(F4[0:32, :], ptf[:, 0:32],
                                                 AF.Exp)
                            for c in range(1, 4):
                                nc.sync.dma_start(
                                    F4[32 * c:32 * (c + 1), :], F4[0:32, :])
                            # idio part 1: features 16032:16384 = cols 160:512
                            nc.sync.dma_start(idio_d[:, 0:352],
                                              raw_t[96:128, 160:512])
                        if jj == 32:
                            # idio part 2: features 16384:16532 = cols 0:148
                            nc.sync.dma_start(idio_d[:, 352:500],
                                              raw_t[0:32, 0:148])

                    # replicate this quad's LT columns to offsets 32/64/96
                    a0, a1 = 16 * 4 * q, min(16 * 4 * (q + 1), NA)
                    if a1 > a0:
                        for c in range(1, 4):
                            nc.sync.dma_start(
                                LT4[32 * c:32 * (c + 1), :, a0:a1],
                                LT4[0:32, :, a0:a1])

                # Software-pipelined quad loop: quad q's FC matmuls + evacs
                # are emitted before quad q-1's transposes/casts so the PE
                # never FIFO-blocks on the previous quad's evac, and the
                # ACT/DVE casts overlap the next quad's matmuls.
                pend = None
                for q in range(n_quads):
                    rr = range(4) if q < 8 else range(1)
                    raw_t = rawp.tile([128, 512], FP, tag="raw",
                                      name=f"raw{q}")
                    fcb_q = rawp.tile([1, 2048], BF, tag="fcb",
                                      name=f"fcb{q}")
                    fw = min(2048 * (q + 1), FH) - 2048 * q
                    nc.sync.dma_start(
                        fcb_q[:, 0:fw], fcbRow_d[:, 2048 * q:2048 * q + fw])
                    for r in rr:
                        jj = 4 * q + r
                        if jj < NPREF:
                            fcw_t = fcw_pref[:, :, jj * 512:(jj + 1) * 512]
                        else:
                            fcw_t = fcwp.tile([128, 4, 512], BF, tag="fcw",
                                              name=f"fcw{jj}")
                            nc.sync.dma_start(
                                fcw_t,
                                fcwT_d[:, jj * 512:(jj + 1) * 512]
                                .rearrange("(ko p) n -> p ko n", p=128),
                            )
                        pfc = pfcp.tile([128, 512], FP, tag="pfc",
                                        name=f"pfc{jj}")
                        mm(pfc[32 * r:32 * (r + 1), :], ones_sb[0:1, :],
                           fcb_q[0:1, 512 * r:512 * (r + 1)],
                           tp=(0, 32 * r), start=True, stop=False)
                        for k in range(4):
                            mm(pfc[32 * r:32 * (r + 1), :],
                               hlast[:, 32 * k:32 * (k + 1)],
                               fcw_t[:, k, :],
                               tp=(0, 32 * r),
                               start=False, stop=(k == 3))
                        s = slice(32 * r, 32 * (r + 1))
                        if r % 2 == 0:
                            nc.scalar.copy(raw_t[s, :], pfc[s, :])
                        else:
                            nc.vector.tensor_copy(raw_t[s, :], pfc[s, :])
                    if pend is not None:
                        emit_lambda(*pend)
                    pend = (q, rr, raw_t)
                emit_lambda(*pend)

                # Sigma per sample: 4 concurrent row-tiled matmuls, evacs
                # into one staging tile, two batched DMAs alternating between
                # the two HWDGE queues (descriptor issue is the bottleneck)
                for b in range(BL):
                    gt4 = gtp.tile([128, 512], BF, tag="gt4")
                    nc.vector.tensor_scalar_mul(gt4[:, 0:500], LT4[:, b, 0:500],
                                                F4[:, b:b + 1])
                    stg = sigw.tile([128, 4, 512], BF, tag="sigstage")
                    for mt in range(4):
                        rows = 128 if mt < 3 else 116
                        ps = ppp.tile([128, 512], FP, tag="pp",
                                      name=f"ps{b}_{mt}")
                        mm(ps[:rows, 0:500],
                           gt4[32 * mt:32 * (mt + 1), 128 * mt:128 * mt + rows],
                           LT4[32 * mt:32 * (mt + 1), b, 0:500],
                           tp=(32 * mt, 0), start=True, stop=True)
                        if mt % 2 == 0:
                            nc.scalar.copy(stg[:rows, mt, 0:500],
                                           ps[:rows, 0:500])
                        else:
                            nc.vector.tensor_copy(stg[:rows, mt, 0:500],
                                                  ps[:rows, 0:500])
                    eng = nc.sync if b % 2 == 0 else nc.scalar
                    eng.dma_start(sigma_d[b], stg)

    nc.compile()
    return nc


# ---------------------------------------------------------------- entry point

def postprocess(results, inputs):
    idx = np.arange(NA)
    out = np.empty((B_FULL, NA, NA), np.float32)
    for core in range(NCORES):
        raw = np.asarray(results[core]["sigma"]).astype(np.float32)
        # [b, p, mt, a] -> rows mt*128+p -> [b, 500, 500]
        sigma = np.ascontiguousarray(
            raw.transpose(0, 2, 1, 3).reshape(BL, 512, 512)[:, :NA, :NA])
        idio = np.exp(np.asarray(results[core]["idio_raw"], np.float32))
        sigma[:, idx, idx] += idio
        out[core * BL:(core + 1) * BL] = sigma
    return out


def kernel(**inputs):
    from concourse.bass_utils import run_bass_kernel_spmd

    prep = host_prep_shared(inputs)
    x = np.asarray(inputs["x"], np.float32)
    in_maps = []
    for core in range(NCORES):
        m = dict(prep)
        m["xT"] = host_prep_x(x[core * BL:(core + 1) * BL])
        in_maps.append(m)

    nc = build_nc()
    res = run_bass_kernel_spmd(nc, in_maps, list(range(NCORES)))
    return postprocess(res.results, inputs)
